# revision 1
# baseline (speedup 1.0000x reference)
"""Trainium2 Bass kernel for nn_ContractExpand (segment_reduce, 5 scales).

out[n, b, l, e] = relu(segsum_r(x)[b, g(l), :] @ (W[n]/r).T + b[n]/r)  broadcast over groups

Strategy (data-parallel over B across 8 cores, 8 batches each):
 - host: append a ones-column to x ([B,800,301], cast bf16); fold bias + the 1/r
   scale into augmented weights WT_aug[n] = [W[n].T/r ; b[n]/r^2]  (the
   ones-column yields a "count row" equal to r via the segment matmul, so
   r * b/r^2 = b/r).  All matmul operands are bf16 (fp32 matmul streams at
   ~4 cyc/col and needs 2 LDWEIGHTS per MM); PSUM accumulation stays f32.
 - device, per batch:
     1. segsum-matmul: one PE pass computes seg_augT[d, g] for ALL 5 scales at
        once (including the r=1 transpose) into a tile-major PSUM tile
        [d, 8 l-tiles x 192]: stationary = natural x tile [100l x d], moving =
        constant 0/1 S_pack [100, 189] -> ONE matmul per l-tile (+ bank splits).
        100-row l-tiles make every scale's group boundaries tile-aligned.
     2. evacuate PSUM -> SBUF bf16 per scale (de-scrambles tile-major to
        scale-major), on ACT/DVE.
     3. main matmul per scale/g-tile: psum[g<=128, 300] = seg_augT.T @ WT_aug,
        3 K-tile accumulation; ReLU evacuates to SBUF f32 on ACT/DVE.
     4. store with the r-fold row replication done by the DMA itself
        (step-0 broadcast source AP -> fully contiguous DRAM writes).
"""

import numpy as np
import ml_dtypes

import concourse.bass as bass
import concourse.tile as tile
from concourse import bacc, mybir
from concourse.bass_utils import run_bass_kernel_spmd

F32 = mybir.dt.float32
BF16 = mybir.dt.bfloat16

R_SCALES = (1, 2, 4, 10, 25)
B, L, D = 64, 800, 300
NCORES = 8
B_LOC = B // NCORES          # 8 batches per core
LT = 100                     # l-tile rows; all scale group sizes align
NT = L // LT                 # 8 l-tiles
SCOLS = [LT // r for r in R_SCALES]                 # 100 50 25 10 4
SCOFF = np.cumsum([0] + SCOLS).tolist()             # s_pack col offsets
SC = SCOFF[-1]                                      # 189
SCP = 190                                           # padded (even) s_pack cols
G = [L // r for r in R_SCALES]                      # 800 400 200 80 32
POFF = np.cumsum([0] + G).tolist()                  # packed seg col offsets
GTOT = POFF[-1]                                     # 1512
DSLICES = [(0, 128), (128, 128), (256, 45)]         # x_aug col K-tiles (45 incl ones)
BANK = 512                                          # psum bank, f32 elems
PBLK = 256                                          # psum cols per l-tile block (8*256 = 4 banks, 2 blocks/bank: no bank-crossing)


def _mm_sched():
    """Segment-matmul schedule per d-tile: (t, s0, w, dst, start, stop).

    One matmul per l-tile covering all 5 scales (s_pack block [100, 189]),
    split where the 192-wide psum block crosses a 512-col bank boundary.
    start/stop go to the first/last matmul into each bank (zero-region
    discipline).
    """
    mms = []
    for t in range(NT):
        w, s0 = SC, 0
        dst = PBLK * t
        while w > 0:
            w1 = min(w, (dst // BANK + 1) * BANK - dst)
            mms.append([t, s0, w1, dst])
            s0 += w1
            dst += w1
            w -= w1
    first, last = {}, {}
    for i, (t, s0, w, dst) in enumerate(mms):
        bk = dst // BANK
        first.setdefault(bk, i)
        last[bk] = i
    return [
        (t, s0, w, dst, i == first[dst // BANK], i == last[dst // BANK])
        for i, (t, s0, w, dst) in enumerate(mms)
    ]


MM_SCHED = _mm_sched()


def build_s_pack():
    s = np.zeros((LT, NT, SCP), np.float32)
    for t in range(NT):
        for si, r in enumerate(R_SCALES):
            for p in range(LT):
                s[p, t, SCOFF[si] + p // r] = 1.0
    return s.astype(ml_dtypes.bfloat16)


def build_wt_aug(W, b):
    out = np.zeros((5, D + 1, D), np.float64)
    for n, r in enumerate(R_SCALES):
        out[n, :D, :] = np.asarray(W[n], np.float64).T / r
        out[n, D, :] = np.asarray(b[n], np.float64) / (r * r)
    return out.astype(ml_dtypes.bfloat16)


def _body(tc, out_ap, x_ap, wt_ap, spk_ap):
    nc = tc.nc
    with (
        tc.tile_pool(name="consts", bufs=1) as consts,
        tc.tile_pool(name="xp", bufs=2) as xp,
        tc.tile_pool(name="segp", bufs=2) as segp,
        tc.tile_pool(name="yp", bufs=2) as yp,
        tc.tile_pool(name="psp", bufs=1, space="PSUM") as psp,
        tc.tile_pool(name="mpsp", bufs=4, space="PSUM") as mpsp,
    ):
        spk_sb = consts.tile([LT, NT, SCP], BF16, name="spk_sb")
        nc.gpsimd.dma_start(out=spk_sb[:, :, :], in_=spk_ap[:, :, :])
        # one coalesced weight DMA per K-tile, on the sync queue so the
        # gpsimd queue stays dedicated to x loads
        wall = []
        for k, (d0, dw) in enumerate(DSLICES):
            w = consts.tile([dw, 5, D], BF16, name=f"wall_{k}")
            nc.sync.dma_start(
                out=w[:, :, :],
                in_=wt_ap[0:5, d0 : d0 + dw, :].transpose([1, 0, 2]),
            )
            wall.append(w)
        wtiles = [[wall[k][:, n, :] for k in range(3)] for n in range(5)]

        def load_x(b):
            x_sb = xp.tile([LT, NT, 304], BF16, name="x_sb", tag="x")
            nc.gpsimd.dma_start(
                out=x_sb[:, :, 0 : D + 1],
                in_=x_ap[b].rearrange("(t p) d -> p t d", p=LT),
            )
            return x_sb

        def segsum_k(x_sb, k):
            d0, dw = DSLICES[k]
            ps = psp.tile([128, 8 * PBLK], F32, name="segps", tag="segps")
            for t, s0, w, dst, start, stop in MM_SCHED:
                nc.tensor.matmul(
                    ps[0:dw, dst : dst + w],
                    x_sb[:, t, d0 : d0 + dw],
                    spk_sb[:, t, s0 : s0 + w],
                    start=start,
                    stop=stop,
                )
            # de-scramble tile-major psum -> scale-major bf16 seg tile
            seg = segp.tile([dw, GTOT], BF16, name=f"seg{k}", tag=f"seg{k}")
            pst = ps[0:dw, :].rearrange("p (t c) -> p t c", c=PBLK)
            for si in range(5):
                w_ = SCOLS[si]
                src = pst[:, :, SCOFF[si] : SCOFF[si] + w_]
                dst_ = seg[:, POFF[si] : POFF[si] + NT * w_].rearrange(
                    "p (t c) -> p t c", t=NT
                )
                if si == 0:
                    nc.scalar.copy(dst_, src)
                else:
                    nc.vector.tensor_copy(dst_, src)
            return seg

        def main_units(b, segs):
            """Yield 15 closures (one per g-tile matmul+relu, with the scale's
            stores attached to its last unit) for interleaved emission."""
            cnt = 0
            for n, r in enumerate(R_SCALES):
                njf, tail = divmod(G[n], 128)
                nj = njf + (1 if tail else 0)
                y = yp.tile([128, nj, D], F32, name=f"y{n}", tag=f"y{n}")
                for j in range(nj):
                    gw = 128 if j < njf else tail

                    def unit(n=n, r=r, j=j, gw=gw, njf=njf, tail=tail, nj=nj, y=y,
                             cnt=cnt, last=(j == nj - 1)):
                        c0 = POFF[n] + 128 * j
                        mp = mpsp.tile([128, BANK], F32, name="mainps", tag="mainps")
                        for k, (d0, dw) in enumerate(DSLICES):
                            nc.tensor.matmul(
                                mp[0:gw, 0:D],
                                segs[k][0:dw, c0 : c0 + gw],
                                wtiles[n][k][:, :],
                                start=(k == 0),
                                stop=(k == 2),
                            )
                        if cnt % 2 == 0:
                            nc.vector.tensor_scalar_max(
                                y[0:gw, j, :], mp[0:gw, 0:D], 0.0
                            )
                        else:
                            nc.scalar.activation(
                                y[0:gw, j, :],
                                mp[0:gw, 0:D],
                                mybir.ActivationFunctionType.Relu,
                            )
                        if last:
                            emit_stores(n, r, njf, tail, nj, y, b)

                    yield unit
                    cnt += 1

        def emit_stores(n, r, njf, tail, nj, y, b):
            # alternate store issue between the SP (sync) and GpSimd DMA
            # queues so descriptor generation isn't serialized on one ring
            dst = out_ap[n, b]
            engs = [nc.sync, nc.gpsimd]
            if r == 1:
                if njf:
                    engs[b % 2].dma_start(
                        out=dst[0 : njf * 128].rearrange("(j p) e -> p j e", p=128),
                        in_=y[:, 0:njf, :],
                    )
                if tail:
                    engs[(b + 1) % 2].dma_start(
                        out=dst[njf * 128 :], in_=y[0:tail, njf, :]
                    )
            else:
                # DMA APs are capped at 3 dims -> one store per g-tile,
                # row-replication via a step-0 broadcast dim on the source.
                for j in range(nj):
                    gw = 128 if j < njf else tail
                    engs[(b + j) % 2].dma_start(
                        out=dst[j * 128 * r : (j * 128 + gw) * r].rearrange(
                            "(p q) e -> p q e", q=r
                        ),
                        in_=y[0:gw, j, :].unsqueeze(1).to_broadcast((gw, r, D)),
                    )

        # software pipeline: segsum of batch b interleaved with main of b-1,
        # 5 main g-tile units after each segsum k-tile (covers the segps
        # bufs=1 evacuation wait with PE work)
        prev_units = None
        x_cur = load_x(0)
        for b in range(B_LOC):
            x_next = load_x(b + 1) if b + 1 < B_LOC else None
            segs = []
            for k in range(3):
                segs.append(segsum_k(x_cur, k))
                if prev_units is not None:
                    for _ in range(5):
                        u = next(prev_units, None)
                        if u is not None:
                            u()
            prev_units = main_units(b, segs)
            x_cur = x_next
        for u in prev_units:
            u()


def build_module():
    nc = bacc.Bacc("TRN2", target_bir_lowering=False, debug=False)
    x = nc.dram_tensor("x", [B_LOC, L, D + 1], BF16, kind="ExternalInput")
    wt = nc.dram_tensor("wt", [5, D + 1, D], BF16, kind="ExternalInput")
    spk = nc.dram_tensor("spk", [LT, NT, SCP], BF16, kind="ExternalInput")
    out = nc.dram_tensor("out", [5, B_LOC, L, D], F32, kind="ExternalOutput")
    with tile.TileContext(nc) as tc:
        _body(tc, out.ap(), x.ap(), wt.ap(), spk.ap())
    nc.compile()
    return nc


_MODULE = None


def _get_module():
    global _MODULE
    if _MODULE is None:
        _MODULE = build_module()
    return _MODULE


def make_in_maps(inputs_c_e, W, b):
    x = np.asarray(inputs_c_e, np.float32)
    x_aug = np.concatenate([x, np.ones((B, L, 1), np.float32)], axis=2).astype(
        ml_dtypes.bfloat16
    )
    wt = build_wt_aug(W, b)
    spk = build_s_pack()
    return [
        {
            "x": np.ascontiguousarray(x_aug[c * B_LOC : (c + 1) * B_LOC]),
            "wt": wt,
            "spk": spk,
        }
        for c in range(NCORES)
    ]


def kernel(inputs_c_e, W, b):
    nc = _get_module()
    in_maps = make_in_maps(inputs_c_e, W, b)
    res = run_bass_kernel_spmd(nc, in_maps, core_ids=list(range(NCORES)))
    out = np.empty((5, B, L, D), np.float32)
    for c in range(NCORES):
        out[:, c * B_LOC : (c + 1) * B_LOC] = res.results[c]["out"]
    return out



# revision 10
# speedup vs baseline: 1.6496x; 1.6496x over previous
"""Trainium2 Bass kernel for nn_ContractExpand (segment_reduce, 5 scales).

out[n, b, l, e] = relu(segsum_r(x)[b, g(l), :] @ (W[n]/r).T + b[n]/r) broadcast
over groups.  Data-parallel over B across 8 cores (8 batches each).

v2 design (PE does ONLY the main matmul, densely packed):
 - host: transpose x to xt[d=301, b, l] (d-major; row 300 = ones column that
   folds the bias: wt_aug[n] = [W[n].T/r ; b[n]/r^2]).  bf16.
 - device:
     * xt is loaded as three k-tiles [dw<=128, 8*800]; for r=1 the main-matmul
       stationary windows slice xt directly (the "transpose" is free).
     * segment sums for r in {2,4,10,25} are strided tensor_reduce ops on the
       DVE (vector) engine, xt -> seg2 -> (seg4, seg10), xt -> seg25, written
       to packed bf16 seg tiles [dw, 5696] (scale-major, batch-major inside).
     * main matmul: 95 stationary windows of <=128 packed group-columns
       spanning ALL 8 batches (99.7% partition fill) x 3 k-tiles, moving =
       wt_aug[n][k] (300 cols, bf16) -> paired PSUM banks.
     * ReLU evacuation PSUM -> one big fp16 y tile, ops alternate across
       ACT / Pool(gpsimd) / DVE; one evac per PSUM bank pair.
     * 13 large fully-contiguous fp16 DMA stores (compact: only the 1512
       unique group rows per batch); the r-fold row replication + f32 upcast
       happens on the host during unshard (pure data movement).
"""

import numpy as np
import ml_dtypes

import concourse.bass as bass
import concourse.tile as tile
from concourse import bacc, mybir
from concourse.bass_utils import run_bass_kernel_spmd

F32 = mybir.dt.float32
F16 = mybir.dt.float16
BF16 = mybir.dt.bfloat16

R_SCALES = (1, 2, 4, 10, 25)
B, L, D = 64, 800, 300
NCORES = 8
B_LOC = B // NCORES                                   # 8 batches per core
G = [L // r for r in R_SCALES]                        # 800 400 200 80 32
G8 = [g * B_LOC for g in G]                           # 6400 3200 1600 640 256
OFF8 = np.cumsum([0] + G8).tolist()                   # out row offsets
GTOT8 = OFF8[-1]                                      # 12096
DSLICES = [(0, 128), (128, 128), (256, 45)]           # xt row k-tiles (45 incl ones)
# seg tile column blocks for scales r>=2 (batch-major inside each block)
SOFF = np.cumsum([0] + G8[1:]).tolist()               # 0 3200 4800 5440 5696
SEGW = SOFF[-1]                                       # 5696

# main-matmul windows: per scale, ceil(G8/128) stationary windows of 128 cols
UNITS = []  # (n, col0_within_scale, gw)
for n in range(5):
    c = 0
    while c < G8[n]:
        gw = min(128, G8[n] - c)
        UNITS.append((n, c, gw))
        c += gw
NU = len(UNITS)                                       # 95

# pair consecutive same-scale full-width units for shared-psum-tile evacs
PAIRS = []  # list of [unit_idx] or [unit_idx, unit_idx]
_i = 0
while _i < NU:
    n, c0, gw = UNITS[_i]
    if _i + 1 < NU and UNITS[_i + 1][0] == n and gw == 128 and UNITS[_i + 1][2] == 128:
        PAIRS.append([_i, _i + 1])
        _i += 2
    else:
        PAIRS.append([_i])
        _i += 1

SCH = 10          # store chunk: units per DMA store
PSUM_BUFS = 4     # pair tiles (2 banks each) in rotation


def build_wt_aug(W, b):
    out = np.zeros((5, D + 1, D), np.float64)
    for n, r in enumerate(R_SCALES):
        out[n, :D, :] = np.asarray(W[n], np.float64).T / r
        out[n, D, :] = np.asarray(b[n], np.float64) / (r * r)
    return out.astype(ml_dtypes.bfloat16)


def _body(tc, out_ap, xt_ap, wt_ap):
    nc = tc.nc
    with (
        tc.tile_pool(name="consts", bufs=1) as consts,
        tc.tile_pool(name="xtp", bufs=1) as xtp,
        tc.tile_pool(name="segp", bufs=1) as segp,
        tc.tile_pool(name="yp", bufs=1) as yp,
        tc.tile_pool(name="psp", bufs=PSUM_BUFS, space="PSUM") as psp,
    ):
        # weights: one DMA per k-tile on the ACT ring (sync/gpsimd do xt)
        wall = []
        for k, (d0, dw) in enumerate(DSLICES):
            w = consts.tile([dw, 5, D], BF16, name=f"wall_{k}")
            nc.scalar.dma_start(
                out=w[:, :, :],
                in_=wt_ap[0:5, d0 : d0 + dw, :].transpose([1, 0, 2]),
            )
            wall.append(w)

        # xt k-tiles, loaded in batch-halves alternating sync/gpsimd rings
        xt = [xtp.tile([128, B_LOC, L], BF16, name=f"xt_{k}") for k in range(3)]
        rings = [nc.sync, nc.gpsimd]
        li = 0
        for h in range(2):
            for k, (d0, dw) in enumerate(DSLICES):
                rings[li % 2].dma_start(
                    out=xt[k][0:dw, h * 4 : h * 4 + 4, :],
                    in_=xt_ap[d0 : d0 + dw, h * 4 : h * 4 + 4, :],
                )
                li += 1

        seg = [segp.tile([128, SEGW], BF16, name=f"seg_{k}") for k in range(3)]
        y = yp.tile([128, NU, D], F16, name="y")

        # ---- seg ops, 2 batches per op, emitted interleaved with main units.
        # DVE (vector) does the strided reduces for r in {2, 10, 25}; the Pool
        # engine (gpsimd, SBUF-only) derives r=4 from seg2 via a strided add.
        def dve_seg_ops():
            with nc.allow_low_precision(reason="bf16 segment sums (tol 2e-2)"):
                for scale in (2, 10, 25):
                    for k, (d0, dw) in enumerate(DSLICES):
                        for b0 in range(0, B_LOC, 2):
                            if scale == 2:
                                src = xt[k][0:dw, b0 : b0 + 2, :].rearrange(
                                    "p b (g r) -> p b g r", r=2
                                )
                                dst = seg[k][0:dw, b0 * 400 : (b0 + 2) * 400]
                            elif scale == 10:
                                src = seg[k][
                                    0:dw, b0 * 400 : (b0 + 2) * 400
                                ].rearrange("p (b g r) -> p b g r", b=2, r=5)
                                dst = seg[k][
                                    0:dw,
                                    SOFF[2] + b0 * 80 : SOFF[2] + (b0 + 2) * 80,
                                ]
                            else:  # 25, straight from xt
                                src = xt[k][0:dw, b0 : b0 + 2, :].rearrange(
                                    "p b (g r) -> p b g r", r=25
                                )
                                dst = seg[k][
                                    0:dw,
                                    SOFF[3] + b0 * 32 : SOFF[3] + (b0 + 2) * 32,
                                ]
                            nc.vector.tensor_reduce(
                                dst.rearrange("p (b g) -> p b g", b=2),
                                src,
                                axis=mybir.AxisListType.X,
                                op=mybir.AluOpType.add,
                            )
                            yield

        def pool_seg_ops():
            with nc.allow_low_precision(reason="bf16 segment sums (tol 2e-2)"):
                for k, (d0, dw) in enumerate(DSLICES):
                    for b0 in range(0, B_LOC, 2):
                        sl = seg[k][0:dw, b0 * 400 : (b0 + 2) * 400].rearrange(
                            "p (g r) -> p g r", r=2
                        )
                        nc.gpsimd.tensor_add(
                            seg[k][
                                0:dw,
                                SOFF[1] + b0 * 200 : SOFF[1] + (b0 + 2) * 200,
                            ],
                            sl[:, :, 0],
                            sl[:, :, 1],
                        )
                        yield

        dve_it = dve_seg_ops()
        pool_it = pool_seg_ops()
        n_segops = 3 * 3 * 4  # DVE ops: 36

        xtf = [
            xt[k][0 : DSLICES[k][1]].rearrange("p b l -> p (b l)") for k in range(3)
        ]

        def stationary(n, k, c0, gw):
            if n == 0:
                return xtf[k][:, c0 : c0 + gw]
            return seg[k][
                0 : DSLICES[k][1], SOFF[n - 1] + c0 : SOFF[n - 1] + c0 + gw
            ]

        # ---- main loop over psum pairs ----
        evac_engines = [nc.scalar, nc.vector]
        pending_stores = []  # units evac'd but not yet stored, same scale
        store_ring = [nc.sync, nc.gpsimd]
        n_stores = 0

        def flush_stores(force=False):
            nonlocal pending_stores, n_stores
            if not pending_stores:
                return
            n0, u0 = pending_stores[0]
            full = [u for (n_, u) in pending_stores if UNITS[u][2] == 128]
            if len(full) != len(pending_stores):
                assert len(pending_stores[0:]) >= 1
            # store any leading run of full-width units as one DMA
            if full and (len(full) >= SCH or force):
                nj = len(full)
                r0 = OFF8[n0] + UNITS[u0][1]
                store_ring[n_stores % 2].dma_start(
                    out=out_ap[r0 : r0 + nj * 128].rearrange(
                        "(j p) e -> p j e", p=128
                    ),
                    in_=y[:, u0 : u0 + nj, :],
                )
                n_stores += 1
                pending_stores = pending_stores[nj:]
            # a trailing non-full unit (r4 tail) is stored alone
            if pending_stores and UNITS[pending_stores[0][1]][2] != 128:
                n_, u_ = pending_stores[0]
                gw = UNITS[u_][2]
                r0 = OFF8[n_] + UNITS[u_][1]
                store_ring[n_stores % 2].dma_start(
                    out=out_ap[r0 : r0 + gw], in_=y[0:gw, u_, :]
                )
                n_stores += 1
                pending_stores = pending_stores[1:]

        emitted_segs = 0
        ui = 0
        for pi, pair in enumerate(PAIRS):
            # interleave seg ops until all are emitted (1 DVE op per pair,
            # 1 Pool op every 3rd pair)
            if emitted_segs < n_segops:
                next(dve_it, None)
                emitted_segs += 1
                if pi % 3 == 0:
                    next(pool_it, None)
            ps = psp.tile([128, 1024], F32, name="mainps", tag="mainps")
            for j, u in enumerate(pair):
                n, c0, gw = UNITS[u]
                for k in range(3):
                    nc.tensor.matmul(
                        ps[0:gw, j * 512 : j * 512 + D],
                        stationary(n, k, c0, gw),
                        wall[k][:, n, :],
                        start=(k == 0),
                        stop=(k == 2),
                    )
            # one ReLU evacuation for the pair
            gw_min = min(UNITS[u][2] for u in pair)
            nj = len(pair)
            u0 = pair[0]
            # Pool/gpsimd cannot access PSUM: evacs go to ACT while the DVE
            # still has seg reduces queued, then alternate ACT/DVE.
            eng = nc.scalar if emitted_segs < n_segops else evac_engines[pi % 2]
            src = ps[0:gw_min, :].rearrange("p (j c) -> p j c", c=512)[:, 0:nj, 0:D]
            dst = y[0:gw_min, u0 : u0 + nj, :]
            if eng is nc.scalar:
                eng.activation(dst, src, mybir.ActivationFunctionType.Relu)
            else:
                eng.tensor_scalar_max(dst, src, 0.0)
            for u in pair:
                pending_stores.append((UNITS[u][0], u))
            ui += nj
            # flush when chunk is big enough or scale ends
            scale_end = (ui == NU) or (UNITS[ui][0] != UNITS[ui - 1][0])
            if len(pending_stores) >= SCH or scale_end:
                flush_stores(force=scale_end)
        for _ in dve_it:
            pass
        for _ in pool_it:
            pass
        flush_stores(force=True)
        assert not pending_stores


def build_module():
    nc = bacc.Bacc("TRN2", target_bir_lowering=False, debug=False)
    xt = nc.dram_tensor("xt", [D + 1, B_LOC, L], BF16, kind="ExternalInput")
    wt = nc.dram_tensor("wt", [5, D + 1, D], BF16, kind="ExternalInput")
    out = nc.dram_tensor("out", [GTOT8, D], F16, kind="ExternalOutput")
    with tile.TileContext(nc) as tc:
        _body(tc, out.ap(), xt.ap(), wt.ap())
    nc.compile()
    return nc


_MODULE = None


def _get_module():
    global _MODULE
    if _MODULE is None:
        _MODULE = build_module()
    return _MODULE


def make_in_maps(inputs_c_e, W, b):
    x = np.asarray(inputs_c_e, np.float32)
    wt = build_wt_aug(W, b)
    # xt[d, b_all, l]; row D is the all-ones bias column
    xt_all = np.empty((D + 1, B, L), np.float32)
    xt_all[:D] = x.transpose(2, 0, 1)
    xt_all[D] = 1.0
    xt_all = xt_all.astype(ml_dtypes.bfloat16)
    return [
        {
            "xt": np.ascontiguousarray(xt_all[:, c * B_LOC : (c + 1) * B_LOC]),
            "wt": wt,
        }
        for c in range(NCORES)
    ]


def expand_core_out(o):
    """[12096, 300] f16 compact rows -> [5, B_LOC, L, D] f32."""
    full = np.empty((5, B_LOC, L, D), np.float32)
    for n, r in enumerate(R_SCALES):
        blk = o[OFF8[n] : OFF8[n + 1]].reshape(B_LOC, G[n], D)
        full[n].reshape(B_LOC, G[n], r, D)[...] = blk[:, :, None, :]
    return full


def kernel(inputs_c_e, W, b):
    nc = _get_module()
    in_maps = make_in_maps(inputs_c_e, W, b)
    res = run_bass_kernel_spmd(nc, in_maps, core_ids=list(range(NCORES)))
    out = np.empty((5, B, L, D), np.float32)
    for c in range(NCORES):
        out[:, c * B_LOC : (c + 1) * B_LOC] = expand_core_out(res.results[c]["out"])
    return out


# revision 11
# speedup vs baseline: 2.6485x; 1.6055x over previous
"""Trainium2 Bass kernel for nn_ContractExpand (segment_reduce, 5 scales).

out[n, b, l, e] = relu(segsum_r(x)[b, g(l), :] @ (W[n]/r).T + b[n]/r) broadcast
over groups.  Data-parallel over B across 8 cores (8 batches each).

v3 design (uniform 128-contraction matmuls at full PE clock):
 - host: transpose x to xt[k, p, b, l] (three zero-PADDED 128-row d-slices;
   row d=300 is the ones column folding the bias: wt_aug[n] = [W[n].T/r ;
   b[n]/r^2 ; 0-pad]).  Sub-128 contraction locks the PE at 1.2GHz and mixed
   PE tile sizes add ~115ns/switch (measured), so every matmul is exactly
   [K=128, M=128, N=300] bf16 -> 125ns streaming at 2.4GHz.
 - device:
     * r=1 stationary windows slice xt directly (transpose is free).
     * seg sums: Pool(gpsimd) computes r2 (strided even+odd add from xt) and
       r4 (from seg2); DVE computes r10 (strided reduce from seg2) and r25
       (reduce from xt).  Packed bf16 seg tiles [128, 5696].
     * main matmul: 95 windows x 3 k-tiles into paired PSUM banks (bufs=4).
     * ReLU evac PSUM -> one fp16 y tile [128, 95, 300]; ACT engine mostly,
       DVE takes late pairs after its seg queue drains.
     * compact fp16 stores (13 contiguous chunks, sync ring, big-to-small);
       r-fold row replication + f32 upcast happens on host during unshard.
 - loads: need-ordered batch chunks, k0/k2+wt on sync ring, k1 on gpsimd
   ring (per-ring transfers serialize; a single dma_start runs ~350GB/s).
"""

import numpy as np
import ml_dtypes

import concourse.bass as bass
import concourse.tile as tile
from concourse import bacc, mybir
from concourse.bass_utils import run_bass_kernel_spmd

F32 = mybir.dt.float32
F16 = mybir.dt.float16
BF16 = mybir.dt.bfloat16

R_SCALES = (1, 2, 4, 10, 25)
B, L, D = 64, 800, 300
DP = 384                                              # padded d (3 x 128)
NCORES = 8
B_LOC = B // NCORES                                   # 8 batches per core
G = [L // r for r in R_SCALES]                        # 800 400 200 80 32
G8 = [g * B_LOC for g in G]                           # 6400 3200 1600 640 256
OFF8 = np.cumsum([0] + G8).tolist()                   # out row offsets
GTOT8 = OFF8[-1]                                      # 12096
# seg tile column blocks for scales r>=2 (batch-major inside each block)
SOFF = np.cumsum([0] + G8[1:]).tolist()               # 0 3200 4800 5440 5696
SEGW = SOFF[-1]                                       # 5696

# main-matmul windows: per scale, ceil(G8/128) windows; stationary is ALWAYS
# 128 cols (the r4 tail window reads 64 junk cols whose out rows aren't
# stored), so every MM is uniform [128, 128, 300].
UNITS = []  # (n, col0_within_scale, gw_store)
for n in range(5):
    c = 0
    while c < G8[n]:
        gw = min(128, G8[n] - c)
        UNITS.append((n, c, gw))
        c += gw
NU = len(UNITS)                                       # 95

PAIRS = []  # consecutive same-scale full-width units share a psum pair tile
_i = 0
while _i < NU:
    n, c0, gw = UNITS[_i]
    if _i + 1 < NU and UNITS[_i + 1][0] == n and gw == 128 and UNITS[_i + 1][2] == 128:
        PAIRS.append([_i, _i + 1])
        _i += 2
    else:
        PAIRS.append([_i])
        _i += 1

SCH = 10          # store chunk: units per DMA store
PSUM_BUFS = 4     # pair tiles (2 banks each)
EVAC_ACT_ONLY = 38  # pairs before this index evac on ACT; later alternate DVE


def build_wt_aug(W, b):
    out = np.zeros((5, DP, D), np.float64)
    for n, r in enumerate(R_SCALES):
        out[n, :D, :] = np.asarray(W[n], np.float64).T / r
        out[n, D, :] = np.asarray(b[n], np.float64) / (r * r)
    return out.astype(ml_dtypes.bfloat16)


def _body(tc, out_ap, xt_ap, wt_ap):
    nc = tc.nc
    with (
        tc.tile_pool(name="consts", bufs=1) as consts,
        tc.tile_pool(name="xtp", bufs=1) as xtp,
        tc.tile_pool(name="segp", bufs=1) as segp,
        tc.tile_pool(name="yp", bufs=1) as yp,
        tc.tile_pool(name="psp", bufs=PSUM_BUFS, space="PSUM") as psp,
    ):
        # weights first on the sync ring (small), then xt chunks in need
        # order: k0/k2 on sync, k1 on gpsimd.
        wall = []
        for k in range(3):
            w = consts.tile([128, 5, D], BF16, name=f"wall_{k}")
            nc.sync.dma_start(
                out=w[:, :, :],
                in_=wt_ap[0:5, k * 128 : (k + 1) * 128, :].transpose([1, 0, 2]),
            )
            wall.append(w)

        xt = [xtp.tile([128, B_LOC, L], BF16, name=f"xt_{k}") for k in range(3)]
        CHUNKS = [(0, 1), (1, 1), (2, 2), (4, 2), (6, 2)]
        for b0, nb in CHUNKS:
            for k in (0, 2, 1):
                ring = nc.gpsimd if k == 1 else nc.sync
                ring.dma_start(
                    out=xt[k][:, b0 : b0 + nb, :],
                    in_=xt_ap[k, :, b0 : b0 + nb, :],
                )

        seg = [segp.tile([128, SEGW], BF16, name=f"seg_{k}") for k in range(3)]
        y = yp.tile([128, NU, D], F16, name="y")

        # ---- seg ops, 2 batches per op, emitted interleaved with the main
        # loop.  Pool: r2 (even+odd strided add from xt) then r4 (from seg2).
        # DVE: r10 (reduce from seg2) and r25 (reduce from xt).
        def pool_seg_ops():
            with nc.allow_low_precision(reason="bf16 segment sums (tol 2e-2)"):
                for b0 in range(0, B_LOC, 2):
                    for k in range(3):
                        src = xt[k][:, b0 : b0 + 2, :].rearrange(
                            "p b (g r) -> p b g r", r=2
                        )
                        dst = seg[k][:, b0 * 400 : (b0 + 2) * 400].rearrange(
                            "p (b g) -> p b g", b=2
                        )
                        nc.gpsimd.tensor_add(dst, src[:, :, :, 0], src[:, :, :, 1])
                        yield
                for b0 in range(0, B_LOC, 2):
                    for k in range(3):
                        s2 = seg[k][:, b0 * 400 : (b0 + 2) * 400].rearrange(
                            "p (b g r) -> p b g r", b=2, r=2
                        )
                        dst = seg[k][
                            :, SOFF[1] + b0 * 200 : SOFF[1] + (b0 + 2) * 200
                        ].rearrange("p (b g) -> p b g", b=2)
                        nc.gpsimd.tensor_add(dst, s2[:, :, :, 0], s2[:, :, :, 1])
                        yield

        def dve_seg_ops():
            with nc.allow_low_precision(reason="bf16 segment sums (tol 2e-2)"):
                for b0 in range(0, B_LOC, 2):
                    for k in range(3):
                        # r10 from seg2 (groups of 5 adjacent seg2 cols)
                        nc.vector.tensor_reduce(
                            seg[k][
                                :, SOFF[2] + b0 * 80 : SOFF[2] + (b0 + 2) * 80
                            ].rearrange("p (b g) -> p b g", b=2),
                            seg[k][:, b0 * 400 : (b0 + 2) * 400].rearrange(
                                "p (b g r) -> p b g r", b=2, r=5
                            ),
                            axis=mybir.AxisListType.X,
                            op=mybir.AluOpType.add,
                        )
                        yield
                        # r25 straight from xt
                        nc.vector.tensor_reduce(
                            seg[k][
                                :, SOFF[3] + b0 * 32 : SOFF[3] + (b0 + 2) * 32
                            ].rearrange("p (b g) -> p b g", b=2),
                            xt[k][:, b0 : b0 + 2, :].rearrange(
                                "p b (g r) -> p b g r", r=25
                            ),
                            axis=mybir.AxisListType.X,
                            op=mybir.AluOpType.add,
                        )
                        yield

        pool_it = pool_seg_ops()
        dve_it = dve_seg_ops()

        def stationary(n, k, c0):
            """Always a 128-col window; precise APs so Tile dep-tracking stays
            chunk-granular (no whole-tile rearrange)."""
            if n == 0:
                b0, b1 = c0 // L, (c0 + 127) // L
                if b0 == b1:
                    return xt[k][:, b0, c0 - b0 * L : c0 - b0 * L + 128]
                return xt[k][:, b0 : b0 + 2, :].rearrange("p b l -> p (b l)")[
                    :, c0 - b0 * L : c0 - b0 * L + 128
                ]
            return seg[k][:, SOFF[n - 1] + c0 : SOFF[n - 1] + c0 + 128]

        # ---- main loop over psum pairs ----
        pending_stores = []
        n_stores = 0

        def flush_stores(force=False):
            nonlocal pending_stores, n_stores
            while pending_stores:
                full = [u for (n_, u) in pending_stores if UNITS[u][2] == 128]
                if full and (len(full) >= SCH or force):
                    n0, u0 = pending_stores[0]
                    nj = len(full)
                    r0 = OFF8[n0] + UNITS[u0][1]
                    nc.sync.dma_start(
                        out=out_ap[r0 : r0 + nj * 128].rearrange(
                            "(j p) e -> p j e", p=128
                        ),
                        in_=y[:, u0 : u0 + nj, :],
                    )
                    n_stores += 1
                    pending_stores = pending_stores[nj:]
                    continue
                if pending_stores and UNITS[pending_stores[0][1]][2] != 128:
                    n_, u_ = pending_stores[0]
                    gw = UNITS[u_][2]
                    r0 = OFF8[n_] + UNITS[u_][1]
                    nc.sync.dma_start(
                        out=out_ap[r0 : r0 + gw], in_=y[0:gw, u_, :]
                    )
                    n_stores += 1
                    pending_stores = pending_stores[1:]
                    continue
                break

        ui = 0
        for pi, pair in enumerate(PAIRS):
            # interleave seg-op emission: one per engine per pair until done
            next(pool_it, None)
            next(dve_it, None)
            ps = psp.tile([128, 1024], F32, name="mainps", tag="mainps")
            for j, u in enumerate(pair):
                n, c0, gw = UNITS[u]
                for k in range(3):
                    nc.tensor.matmul(
                        ps[0:128, j * 512 : j * 512 + D],
                        stationary(n, k, c0),
                        wall[k][:, n, :],
                        start=(k == 0),
                        stop=(k == 2),
                    )
            nj = len(pair)
            u0 = pair[0]
            gw_min = min(UNITS[u][2] for u in pair)
            src = ps[0:gw_min, :].rearrange("p (j c) -> p j c", c=512)[:, 0:nj, 0:D]
            dst = y[0:gw_min, u0 : u0 + nj, :]
            if pi < EVAC_ACT_ONLY or pi % 2 == 0:
                nc.scalar.activation(dst, src, mybir.ActivationFunctionType.Relu)
            else:
                nc.vector.tensor_scalar_max(dst, src, 0.0)
            for u in pair:
                pending_stores.append((UNITS[u][0], u))
            ui += nj
            scale_end = (ui == NU) or (UNITS[ui][0] != UNITS[ui - 1][0])
            if len(pending_stores) >= SCH or scale_end:
                flush_stores(force=scale_end)
        for _ in pool_it:
            pass
        for _ in dve_it:
            pass
        flush_stores(force=True)
        assert not pending_stores


def build_module():
    nc = bacc.Bacc("TRN2", target_bir_lowering=False, debug=False)
    xt = nc.dram_tensor("xt", [3, 128, B_LOC, L], BF16, kind="ExternalInput")
    wt = nc.dram_tensor("wt", [5, DP, D], BF16, kind="ExternalInput")
    out = nc.dram_tensor("out", [GTOT8, D], F16, kind="ExternalOutput")
    with tile.TileContext(nc) as tc:
        _body(tc, out.ap(), xt.ap(), wt.ap())
    nc.compile()
    return nc


_MODULE = None


def _get_module():
    global _MODULE
    if _MODULE is None:
        _MODULE = build_module()
    return _MODULE


def make_in_maps(inputs_c_e, W, b):
    x = np.asarray(inputs_c_e, np.float32)
    wt = build_wt_aug(W, b)
    # xt[(k p), b_all, l]; row d=300 is the ones bias column, rest zero-pad
    xt_all = np.zeros((DP, B, L), np.float32)
    xt_all[:D] = x.transpose(2, 0, 1)
    xt_all[D] = 1.0
    xt_all = xt_all.astype(ml_dtypes.bfloat16).reshape(3, 128, B, L)
    return [
        {
            "xt": np.ascontiguousarray(xt_all[:, :, c * B_LOC : (c + 1) * B_LOC]),
            "wt": wt,
        }
        for c in range(NCORES)
    ]


def expand_core_out(o):
    """[12096, 300] f16 compact rows -> [5, B_LOC, L, D] f32."""
    full = np.empty((5, B_LOC, L, D), np.float32)
    for n, r in enumerate(R_SCALES):
        blk = o[OFF8[n] : OFF8[n + 1]].reshape(B_LOC, G[n], D)
        full[n].reshape(B_LOC, G[n], r, D)[...] = blk[:, :, None, :]
    return full


def kernel(inputs_c_e, W, b):
    nc = _get_module()
    in_maps = make_in_maps(inputs_c_e, W, b)
    res = run_bass_kernel_spmd(nc, in_maps, core_ids=list(range(NCORES)))
    out = np.empty((5, B, L, D), np.float32)
    for c in range(NCORES):
        out[:, c * B_LOC : (c + 1) * B_LOC] = expand_core_out(res.results[c]["out"])
    return out


# revision 16
# speedup vs baseline: 3.1448x; 1.1874x over previous
"""Trainium2 Bass kernel for nn_ContractExpand (segment_reduce, 5 scales).

out[n, b, l, e] = relu(segsum_r(x)[b, g(l), :] @ (W[n]/r).T + b[n]/r) broadcast
over groups.  Data-parallel over B across 8 cores (8 batches each).

v3 design (uniform 128-contraction matmuls at full PE clock):
 - host: transpose x to xt[k, p, b, l] (three zero-PADDED 128-row d-slices;
   row d=300 is the ones column folding the bias: wt_aug[n] = [W[n].T/r ;
   b[n]/r^2 ; 0-pad]).  Sub-128 contraction locks the PE at 1.2GHz and mixed
   PE tile sizes add ~115ns/switch (measured), so every matmul is exactly
   [K=128, M=128, N=300] bf16 -> 125ns streaming at 2.4GHz.
 - device:
     * r=1 stationary windows slice xt directly (transpose is free).
     * seg sums: Pool(gpsimd) computes r2 (strided even+odd add from xt) and
       r4 (from seg2); DVE computes r10 (strided reduce from seg2) and r25
       (reduce from xt).  Packed bf16 seg tiles [128, 5696].
     * main matmul: 95 windows x 3 k-tiles into paired PSUM banks (bufs=4).
     * ReLU evac PSUM -> one fp16 y tile [128, 95, 300]; ACT engine mostly,
       DVE takes late pairs after its seg queue drains.
     * compact fp16 stores (13 contiguous chunks, sync ring, big-to-small);
       r-fold row replication + f32 upcast happens on host during unshard.
 - loads: need-ordered batch chunks, k0/k2+wt on sync ring, k1 on gpsimd
   ring (per-ring transfers serialize; a single dma_start runs ~350GB/s).
"""

import numpy as np
import ml_dtypes

import concourse.bass as bass
import concourse.tile as tile
from concourse import bacc, mybir
from concourse.bass_utils import run_bass_kernel_spmd

F32 = mybir.dt.float32
F16 = mybir.dt.float16
BF16 = mybir.dt.bfloat16

R_SCALES = (1, 2, 4, 10, 25)
B, L, D = 64, 800, 300
DP = 384                                              # padded d (3 x 128)
NCORES = 8
B_LOC = B // NCORES                                   # 8 batches per core
G = [L // r for r in R_SCALES]                        # 800 400 200 80 32
G8 = [g * B_LOC for g in G]                           # 6400 3200 1600 640 256
OFF8 = np.cumsum([0] + G8).tolist()                   # out row offsets
GTOT8 = OFF8[-1]                                      # 12096
# seg tile column blocks for scales r>=2 (batch-major inside each block)
SOFF = np.cumsum([0] + G8[1:]).tolist()               # 0 3200 4800 5440 5696
SEGW = SOFF[-1]                                       # 5696

# main-matmul windows: per scale, ceil(G8/128) windows; stationary is ALWAYS
# 128 cols (the r4 tail window reads 64 junk cols whose out rows aren't
# stored), so every MM is uniform [128, 128, 300].
UNITS = []  # (n, col0_within_scale, gw_store)
for n in range(5):
    c = 0
    while c < G8[n]:
        gw = min(128, G8[n] - c)
        UNITS.append((n, c, gw))
        c += gw
NU = len(UNITS)                                       # 95

PAIRS = []  # consecutive same-scale full-width units share a psum pair tile
_i = 0
while _i < NU:
    n, c0, gw = UNITS[_i]
    if _i + 1 < NU and UNITS[_i + 1][0] == n and gw == 128 and UNITS[_i + 1][2] == 128:
        PAIRS.append([_i, _i + 1])
        _i += 2
    else:
        PAIRS.append([_i])
        _i += 1

SCH = 10          # store chunk: units per DMA store
PSUM_BUFS = 4     # pair tiles (2 banks each)
EVAC_ACT_ONLY = 38  # pairs before this index evac on ACT; later alternate DVE


def build_wt_aug(W, b):
    out = np.zeros((5, DP, D), np.float64)
    for n, r in enumerate(R_SCALES):
        out[n, :D, :] = np.asarray(W[n], np.float64).T / r
        out[n, D, :] = np.asarray(b[n], np.float64) / (r * r)
    return out.astype(ml_dtypes.bfloat16)


def _body(tc, out_ap, xt_ap, wt_ap):
    nc = tc.nc
    with (
        tc.tile_pool(name="consts", bufs=1) as consts,
        tc.tile_pool(name="xtp", bufs=1) as xtp,
        tc.tile_pool(name="segp", bufs=1) as segp,
        tc.tile_pool(name="yp", bufs=1) as yp,
        tc.tile_pool(name="psp", bufs=PSUM_BUFS, space="PSUM") as psp,
    ):
        # weights first on the sync ring (small), then xt chunks in need
        # order: k0/k2 on sync, k1 on gpsimd.
        wall = []
        for k in range(3):
            w = consts.tile([128, 5, D], BF16, name=f"wall_{k}")
            nc.sync.dma_start(
                out=w[:, :, :],
                in_=wt_ap[0:5, k * 128 : (k + 1) * 128, :].transpose([1, 0, 2]),
            )
            wall.append(w)

        xt = [xtp.tile([128, B_LOC, L], BF16, name=f"xt_{k}") for k in range(3)]

        def load_chunk(b0, nb):
            for k in (0, 2, 1):
                ring = nc.gpsimd if k == 1 else nc.sync
                ring.dma_start(
                    out=xt[k][:, b0 : b0 + nb, :],
                    in_=xt_ap[k, :, b0 : b0 + nb, :],
                )

        # only b0/b1 upfront; later chunks are emitted lazily inside the main
        # loop (DMA completion tracking is ring-ordered, so emitting all loads
        # upfront makes early consumers wait on the whole ring chain)
        load_chunk(0, 1)
        load_chunk(1, 1)
        # chunk (b0, nb) must be emitted before its first consumer: the Pool
        # r2 op for batch-pair b0 is emitted at pair 3*(b0//2)
        LAZY_LOADS = {2: (2, 2), 5: (4, 2), 8: (6, 2)}

        seg = [segp.tile([128, SEGW], BF16, name=f"seg_{k}") for k in range(3)]
        y = yp.tile([128, NU, D], F16, name="y")

        # ---- seg ops, 2 batches per op, emitted interleaved with the main
        # loop.  Pool: r2 (even+odd strided add from xt) then r4 (from seg2).
        # DVE: r10 (reduce from seg2) and r25 (reduce from xt).
        def pool_seg_ops():
            with nc.allow_low_precision(reason="bf16 segment sums (tol 2e-2)"):
                for b0 in range(0, B_LOC, 2):
                    for k in range(3):
                        src = xt[k][:, b0 : b0 + 2, :].rearrange(
                            "p b (g r) -> p b g r", r=2
                        )
                        dst = seg[k][:, b0 * 400 : (b0 + 2) * 400].rearrange(
                            "p (b g) -> p b g", b=2
                        )
                        nc.gpsimd.tensor_add(dst, src[:, :, :, 0], src[:, :, :, 1])
                        yield
                for b0 in range(0, B_LOC, 2):
                    for k in range(3):
                        s2 = seg[k][:, b0 * 400 : (b0 + 2) * 400].rearrange(
                            "p (b g r) -> p b g r", b=2, r=2
                        )
                        dst = seg[k][
                            :, SOFF[1] + b0 * 200 : SOFF[1] + (b0 + 2) * 200
                        ].rearrange("p (b g) -> p b g", b=2)
                        nc.gpsimd.tensor_add(dst, s2[:, :, :, 0], s2[:, :, :, 1])
                        yield

        def dve_seg_ops():
            with nc.allow_low_precision(reason="bf16 segment sums (tol 2e-2)"):
                for b0 in range(0, B_LOC, 2):
                    for k in range(3):
                        # r10 from seg2 (groups of 5 adjacent seg2 cols)
                        nc.vector.tensor_reduce(
                            seg[k][
                                :, SOFF[2] + b0 * 80 : SOFF[2] + (b0 + 2) * 80
                            ].rearrange("p (b g) -> p b g", b=2),
                            seg[k][:, b0 * 400 : (b0 + 2) * 400].rearrange(
                                "p (b g r) -> p b g r", b=2, r=5
                            ),
                            axis=mybir.AxisListType.X,
                            op=mybir.AluOpType.add,
                        )
                        yield
                        # r25 straight from xt
                        nc.vector.tensor_reduce(
                            seg[k][
                                :, SOFF[3] + b0 * 32 : SOFF[3] + (b0 + 2) * 32
                            ].rearrange("p (b g) -> p b g", b=2),
                            xt[k][:, b0 : b0 + 2, :].rearrange(
                                "p b (g r) -> p b g r", r=25
                            ),
                            axis=mybir.AxisListType.X,
                            op=mybir.AluOpType.add,
                        )
                        yield

        pool_it = pool_seg_ops()
        dve_it = dve_seg_ops()

        def stationary(n, k, c0):
            """Always a 128-col window; precise APs so Tile dep-tracking stays
            chunk-granular (no whole-tile rearrange)."""
            if n == 0:
                b0, b1 = c0 // L, (c0 + 127) // L
                if b0 == b1:
                    return xt[k][:, b0, c0 - b0 * L : c0 - b0 * L + 128]
                return xt[k][:, b0 : b0 + 2, :].rearrange("p b l -> p (b l)")[
                    :, c0 - b0 * L : c0 - b0 * L + 128
                ]
            return seg[k][:, SOFF[n - 1] + c0 : SOFF[n - 1] + c0 + 128]

        # ---- main loop over psum pairs ----
        # big early chunks store on the sync ring; the last few small chunks
        # go to the gpsimd ring (free after its seg adds) to cut the tail
        pending_stores = []
        n_stores = 0

        def flush_stores(force=False, late=False):
            nonlocal pending_stores, n_stores
            ring = nc.gpsimd if late else nc.sync
            while pending_stores:
                full = [u for (n_, u) in pending_stores if UNITS[u][2] == 128]
                if full and (len(full) >= SCH or force):
                    n0, u0 = pending_stores[0]
                    nj = len(full)
                    r0 = OFF8[n0] + UNITS[u0][1]
                    ring.dma_start(
                        out=out_ap[r0 : r0 + nj * 128].rearrange(
                            "(j p) e -> p j e", p=128
                        ),
                        in_=y[:, u0 : u0 + nj, :],
                    )
                    n_stores += 1
                    pending_stores = pending_stores[nj:]
                    continue
                if pending_stores and UNITS[pending_stores[0][1]][2] != 128:
                    n_, u_ = pending_stores[0]
                    gw = UNITS[u_][2]
                    r0 = OFF8[n_] + UNITS[u_][1]
                    ring.dma_start(
                        out=out_ap[r0 : r0 + gw], in_=y[0:gw, u_, :]
                    )
                    n_stores += 1
                    pending_stores = pending_stores[1:]
                    continue
                break

        ui = 0
        for pi, pair in enumerate(PAIRS):
            if pi in LAZY_LOADS:
                load_chunk(*LAZY_LOADS[pi])
            # interleave seg-op emission: one per engine per pair until done
            next(pool_it, None)
            next(dve_it, None)
            ps = psp.tile([128, 1024], F32, name="mainps", tag="mainps")
            for j, u in enumerate(pair):
                n, c0, gw = UNITS[u]
                for k in range(3):
                    nc.tensor.matmul(
                        ps[0:128, j * 512 : j * 512 + D],
                        stationary(n, k, c0),
                        wall[k][:, n, :],
                        start=(k == 0),
                        stop=(k == 2),
                    )
            nj = len(pair)
            u0 = pair[0]
            gw_min = min(UNITS[u][2] for u in pair)
            src = ps[0:gw_min, :].rearrange("p (j c) -> p j c", c=512)[:, 0:nj, 0:D]
            dst = y[0:gw_min, u0 : u0 + nj, :]
            if pi < EVAC_ACT_ONLY or pi % 2 == 0:
                nc.scalar.activation(dst, src, mybir.ActivationFunctionType.Relu)
            else:
                nc.vector.tensor_scalar_max(dst, src, 0.0)
            for u in pair:
                pending_stores.append((UNITS[u][0], u))
            ui += nj
            scale_end = (ui == NU) or (UNITS[ui][0] != UNITS[ui - 1][0])
            if len(pending_stores) >= SCH or scale_end:
                flush_stores(force=scale_end, late=(ui > 80))
        for _ in pool_it:
            pass
        for _ in dve_it:
            pass
        flush_stores(force=True, late=True)
        assert not pending_stores


def build_module():
    nc = bacc.Bacc("TRN2", target_bir_lowering=False, debug=False)
    xt = nc.dram_tensor("xt", [3, 128, B_LOC, L], BF16, kind="ExternalInput")
    wt = nc.dram_tensor("wt", [5, DP, D], BF16, kind="ExternalInput")
    out = nc.dram_tensor("out", [GTOT8, D], F16, kind="ExternalOutput")
    with tile.TileContext(nc) as tc:
        _body(tc, out.ap(), xt.ap(), wt.ap())
    nc.compile()
    return nc


_MODULE = None


def _get_module():
    global _MODULE
    if _MODULE is None:
        _MODULE = build_module()
    return _MODULE


def make_in_maps(inputs_c_e, W, b):
    x = np.asarray(inputs_c_e, np.float32)
    wt = build_wt_aug(W, b)
    # xt[(k p), b_all, l]; row d=300 is the ones bias column, rest zero-pad
    xt_all = np.zeros((DP, B, L), np.float32)
    xt_all[:D] = x.transpose(2, 0, 1)
    xt_all[D] = 1.0
    xt_all = xt_all.astype(ml_dtypes.bfloat16).reshape(3, 128, B, L)
    return [
        {
            "xt": np.ascontiguousarray(xt_all[:, :, c * B_LOC : (c + 1) * B_LOC]),
            "wt": wt,
        }
        for c in range(NCORES)
    ]


def expand_core_out(o):
    """[12096, 300] f16 compact rows -> [5, B_LOC, L, D] f32."""
    full = np.empty((5, B_LOC, L, D), np.float32)
    for n, r in enumerate(R_SCALES):
        blk = o[OFF8[n] : OFF8[n + 1]].reshape(B_LOC, G[n], D)
        full[n].reshape(B_LOC, G[n], r, D)[...] = blk[:, :, None, :]
    return full


def kernel(inputs_c_e, W, b):
    nc = _get_module()
    in_maps = make_in_maps(inputs_c_e, W, b)
    res = run_bass_kernel_spmd(nc, in_maps, core_ids=list(range(NCORES)))
    out = np.empty((5, B, L, D), np.float32)
    for c in range(NCORES):
        out[:, c * B_LOC : (c + 1) * B_LOC] = expand_core_out(res.results[c]["out"])
    return out
